# revision 56
# baseline (speedup 1.0000x reference)
"""Trainium2 Bass kernel for nn_MultiHeadAttention_7584912245188.

Reference computes (no softmax!):
    qkv = x @ Wqkv + bqkv ; split q,k,v ; per head: y = (q k^T / sqrt(D)) v
    out = y @ Wff + bff

No softmax => attention is linear and reassociates: (Q K^T) V = Q (K^T V).
With X_aug = [X | 1] ([N, 97]) and G = X_aug^T X_aug ([97, 97]) the module
collapses to out = X_aug @ Wfin computed on device as:
    R = G @ Qcat                  [97, 576]  (2 matmuls; Q_h = s Wv_h Wff_h)
    Wfin = sum_h P_h R_h + bff    [97, 96]   (7-matmul group; P_h = Wq_h Wk_h^T)
    out rows {8p+j} = X @ Wfin    (8 matmuls via transposed X)
P/Q are host-folded: pre-multiplying Wq Wk^T keeps the post-Gram dependency
chain at three matmul stages - each extra stage costs ~570ns of cross-engine
semaphore latency regardless of size. O(N*E^2) instead of O(N^2*D).

Sharding (8 cores): core c -> (batch b = c//2, half h = c%2). Each core
computes the full-batch Gram redundantly (cheaper than a collective) and
writes its own half of the rows.

Schedule (fixed DMA costs dominate: HWDGE issue ~630 + 650 DGE delay, SWDGE
prep ~1040 + 650, 900ns completion sem, shared ~360GB/s DMA engines):
  - near half rides the first sync-HWDGE slot in fp16 (Gram + transposes);
    the far half is QUANTIZED TO FP8 (it only feeds the Gram; G is
    diagonally dominated, measured end-to-end rel err ~6e-3) and rides the
    Pool SWDGE path whose descriptor prep overlaps the first transfer.
  - the ~230KB P/Q weights ride the activation-queue HWDGE slot; they
    arrive just before the R matmul needs them (off the critical path).
  - a chain of tiny matmuls warms the PE p-state ramp before the real work.
  - X^T comes from 8 PE transposes that fill the PE idle gap between the
    Gram close and the R matmul; their identity operand is built on-chip by
    the idle Pool engine (a DMA'd identity lands too late for that gap).
  - both result halves leave as one HWDGE store (a second store's 625ns
    issue would serialize behind the first and cost more than the extra
    273ns of transfer).
"""

import numpy as np
from contextlib import ExitStack

import ml_dtypes
import concourse.bass as bass
import concourse.tile as tile
from concourse import bacc, mybir
from concourse import bass_utils
from concourse.masks import make_identity

B, N, E = 4, 2048, 96
H = 6
D = E // H            # 16
EA = E + 1            # 97 (augmented ones column)
NH = N // 2           # 1024 rows per half
NCH = 8               # row chunks per half (chunk j = rows {8p + j})
SCALE = float(D) ** -0.5
F32 = mybir.dt.float32
F16 = mybir.dt.float16
F8 = mybir.dt.float8e4
NP_F8 = ml_dtypes.float8_e4m3

N_WARM = 12           # PE p-state warmup matmuls
WARM_COLS = 128

# wpack (fp16) column layout: PcatT (6x[97,97]) | Qcat (6x[97,96]) | identity
C_P = 0               # P_h^T = (Wq_aug_h Wk_aug_h^T)^T, host-precomputed
C_Q = 582             # Q_h = SCALE * Wv_aug_h Wff_h
WCOLS = 1158
# wq2 (fp16, 1 partition): onehot row (1.0 at col 96) | bff
C_OH = 0
C_BF = 97
WQCOLS = 193

N_CORES = 8

_NC_CACHE = {}
LAST_RESULTS = None


def _build_nc():
    nc = bacc.Bacc(
        "TRN2", target_bir_lowering=False, debug=False, num_devices=N_CORES,
    )
    xa = nc.dram_tensor("xa", [NH, EA], F16, kind="ExternalInput").ap()
    xb = nc.dram_tensor("xb", [NH, EA], F8, kind="ExternalInput").ap()
    wpi = nc.dram_tensor("wpack", [128, WCOLS], F16, kind="ExternalInput").ap()
    wqi = nc.dram_tensor("wq2", [1, WQCOLS], F16, kind="ExternalInput").ap()
    outd = nc.dram_tensor("out", [128, 8 * E], F16, kind="ExternalOutput").ap()

    with tile.TileContext(nc) as tc, ExitStack() as ctx:
        consts = ctx.enter_context(tc.tile_pool(name="consts", bufs=1))
        big = ctx.enter_context(tc.tile_pool(name="big", bufs=1))
        small = ctx.enter_context(tc.tile_pool(name="small", bufs=1))
        outp = ctx.enter_context(tc.tile_pool(name="outp", bufs=1))
        ps_gw = ctx.enter_context(tc.tile_pool(name="ps_gw", bufs=1, space="PSUM"))
        ps_r = ctx.enter_context(tc.tile_pool(name="ps_r", bufs=2, space="PSUM"))
        ps_t = ctx.enter_context(tc.tile_pool(name="ps_t", bufs=2, space="PSUM"))
        ps_o = ctx.enter_context(tc.tile_pool(name="ps_o", bufs=2, space="PSUM"))

        # --- near half fp16 on the first sync-HWDGE slot; far half fp8 on
        # the Pool SWDGE path (its prep overlaps xa's transfer); weights on
        # the activation HWDGE queue, Wk|Wvff|identity first
        XA = big.tile([128, NCH, EA], F16)
        nc.sync.dma_start(out=XA[:], in_=xa.rearrange("(p j) e -> p j e", j=NCH))
        XB = big.tile([128, NCH, EA], F8)
        nc.gpsimd.dma_start(out=XB[:], in_=xb.rearrange("(p j) e -> p j e", j=NCH))
        wp = consts.tile([128, WCOLS], F16)
        nc.scalar.dma_start(out=wp[:], in_=wpi)
        wq2 = consts.tile([1, WQCOLS], F16)
        nc.scalar.dma_start(out=wq2[:], in_=wqi)
        # identity for the PE transposes, built by the (otherwise idle) Pool
        # engine right after the xb descriptor prep - ready ~2us, long
        # before the weights DMA would deliver it
        id_sb = consts.tile([128, 128], F16)
        make_identity(nc, id_sb[:])

        # --- PE p-state warmup: keep the tensor engine busy from ~0.7us so
        # the ramp model is past the slow state when the real matmuls start
        wu = small.tile([1, WARM_COLS], F16)
        nc.vector.memset(wu[:], 0.0)
        wu_ps = ps_o.tile([1, WARM_COLS], F32, tag="og", name="warm")
        for _ in range(N_WARM):
            nc.tensor.matmul(
                wu_ps[:], lhsT=wu[0:1, 0:1], rhs=wu[:], start=True, stop=True
            )

        # --- G = X_aug^T X_aug, one 16-matmul PSUM accumulation group
        # (near half first - it arrives first)
        g_ps = ps_gw.tile([EA, EA], F32, tag="gw", name="g")
        for c in range(NCH):
            xc = XA[:, c, :]
            nc.tensor.matmul(g_ps[:], lhsT=xc, rhs=xc, start=(c == 0), stop=False)
        for c in range(NCH):
            xc = XB[:, c, :]
            nc.tensor.matmul(
                g_ps[:], lhsT=xc, rhs=xc, start=False, stop=(c == NCH - 1)
            )
        g_h = small.tile([EA, EA], F16)
        nc.vector.tensor_copy(out=g_h[:], in_=g_ps[:])

        # --- 8 PE transposes of the near half (PE is idle while the chain
        # copies run); staging copies run off the critical chain
        XT = big.tile([EA, NCH, 128], F16)
        pts = []
        for grp in range(2):
            pt = ps_t.tile([EA, 4, 128], F16, tag="pt", name=f"pt{grp}")
            for j in range(4):
                nc.tensor.transpose(
                    out=pt[:, j, :], in_=XA[:, 4 * grp + j, :],
                    identity=id_sb[:],
                )
            pts.append(pt)

        # --- R = G @ Qcat in two free-dim halves (PSUM bank limit); r0
        # staged by DVE, r1 by Act so the copies overlap
        r_h = small.tile([EA, H * E], F16)
        HW2 = H * E // 2
        rps = []
        for half in range(2):
            r_ps = ps_r.tile([EA, HW2], F32, tag="r", name=f"r{half}")
            nc.tensor.matmul(
                r_ps[:], lhsT=g_h[:],
                rhs=wp[0:EA, C_Q + HW2 * half : C_Q + HW2 * (half + 1)],
                start=True, stop=True,
            )
            rps.append(r_ps)
        nc.vector.tensor_copy(out=r_h[:, 0:HW2], in_=rps[0][:])
        nc.scalar.copy(out=r_h[:, HW2 : H * E], in_=rps[1][:])
        nc.vector.tensor_copy(out=XT[:, 0:4, :], in_=pts[0][:])
        nc.vector.tensor_copy(out=XT[:, 4:8, :], in_=pts[1][:])

        # --- Wfin = sum_h P_h R_h + e_last bff^T (one accum group, PSUM
        # bank shared with G - dead after g_h)
        wf_ps = ps_gw.tile([EA, E], F32, tag="gw", name="wf")
        for h in range(H):
            nc.tensor.matmul(
                wf_ps[:],
                lhsT=wp[0:EA, C_P + EA * h : C_P + EA * (h + 1)],
                rhs=r_h[:, E * h : E * (h + 1)],
                start=(h == 0), stop=False,
            )
        nc.tensor.matmul(
            wf_ps[:],
            lhsT=wq2[0:1, C_OH : C_OH + EA],
            rhs=wq2[0:1, C_BF : C_BF + E],
            start=False, stop=True,
        )
        wf_h = small.tile([EA, E], F16)
        nc.vector.tensor_copy(out=wf_h[:], in_=wf_ps[:])

        # --- finals: out rows {8p+j} = X_chunk @ Wfin; each half leaves as
        # its own HWDGE store so the issues/transfers overlap
        osb = outp.tile([128, 2, 4 * E], F16)
        for g in range(2):
            og = ps_o.tile([128, 4, E], F32, tag="og", name=f"og{g}")
            for j4 in range(4):
                nc.tensor.matmul(
                    og[:, j4, :], lhsT=XT[:, 4 * g + j4, :], rhs=wf_h[:],
                    start=True, stop=True,
                )
            cp = nc.vector.tensor_copy if g == 0 else nc.scalar.copy
            cp(out=osb[:, g, :], in_=og[:].rearrange("p a b -> p (a b)"))
        nc.sync.dma_start(out=outd, in_=osb[:].rearrange("p a b -> p (a b)"))

    nc.compile()
    return nc


def get_nc():
    if "nc" not in _NC_CACHE:
        _NC_CACHE["nc"] = _build_nc()
    return _NC_CACHE["nc"]


def _host_weights(Wqkv, bqkv, Wff, bff):
    waug = np.concatenate(
        [np.asarray(Wqkv, np.float64), np.asarray(bqkv, np.float64)[None, :]], axis=0
    )
    Wq, Wk, Wv = waug[:, 0:E], waug[:, E : 2 * E], waug[:, 2 * E : 3 * E]
    Wff = np.asarray(Wff, np.float64)
    wp = np.zeros((128, WCOLS), np.float16)
    wq2 = np.zeros((1, WQCOLS), np.float16)
    for h in range(H):
        hd = slice(h * D, (h + 1) * D)
        wp[0:EA, C_P + EA * h : C_P + EA * (h + 1)] = (
            Wq[:, hd] @ Wk[:, hd].T
        ).T.astype(np.float16)
        wp[0:EA, C_Q + E * h : C_Q + E * (h + 1)] = (
            SCALE * (Wv[:, hd] @ Wff[hd, :])
        ).astype(np.float16)
    wq2[0, C_OH + E] = 1.0
    wq2[0, C_BF : C_BF + E] = np.asarray(bff, np.float16)
    return {"wpack": wp, "wq2": wq2}


def make_in_maps(x, Wqkv, bqkv, Wff, bff):
    x = np.asarray(x, np.float32)
    w = _host_weights(Wqkv, bqkv, Wff, bff)
    x16 = x.astype(np.float16)
    in_maps = []
    for c in range(N_CORES):
        b, h = divmod(c, 2)
        mine = x16[b, h * NH : (h + 1) * NH]
        other = x16[b, (1 - h) * NH : (2 - h) * NH]
        xa = np.ones((NH, EA), np.float16)
        xa[:, 0:E] = mine
        xbm = np.ones((NH, EA), np.float16)
        xbm[:, 0:E] = other
        m = {"xa": xa, "xb": xbm.astype(NP_F8)}
        m.update(w)
        in_maps.append(m)
    return in_maps


def assemble(results):
    out = np.empty((B, N, E), np.float32)
    for c in range(N_CORES):
        b, h = divmod(c, 2)
        half = results[c]["out"].reshape(128, 8, E).astype(np.float32)
        out[b, h * NH : (h + 1) * NH] = half.reshape(NH, E)
    return out


def kernel(x, Wqkv, bqkv, Wff, bff):
    global LAST_RESULTS
    nc = get_nc()
    in_maps = make_in_maps(x, Wqkv, bqkv, Wff, bff)
    res = bass_utils.run_bass_kernel_spmd(
        nc, in_maps, core_ids=list(range(N_CORES))
    )
    LAST_RESULTS = res
    return assemble(res.results)
